# revision 12
# baseline (speedup 1.0000x reference)
"""Correlation kernel (max_disp=1, 9 offsets) for Trainium2, 8 NeuronCores.

Computation (per batch b):
    out[dx*3+dy, i, j] = mean_c( x1[c,i,j] * pad(x2)[c, i+dy, j+dx] )
with B=8, C=512, H=W=96, pad=1 on each spatial side.

Sharding: data-parallel over batch - core b handles batch b. No collectives.

Per-core strategy (v4, TensorE band-matmul):
  - Host casts inputs to bf16 (tolerance is 2e-2; bf16 dot error ~5e-3),
    pre-pads x2 to [4ct, 128c, 98, 98], and pre-tiles x1 patch-major as
    [6 pair, 4 ct, 128 c, 12 q, 128 m]  (patch = 8x16 interior pixels,
    m = r*16+s; q = halfbi*6+bj; pair of bi-rows per DMA strip).
  - Device: for each 8x16 interior patch, LDWEIGHTS x1-patch (lhsT
    [c=128, m=128]) and matmul against the 10x18 halo of padded x2
    (rhs [c=128, 180]), accumulating over the 4 c-tiles in PSUM.
    psum[m, n] = sum_c x1[c, pix m] * x2p[c, halo pix n]: all 9 offsets
    of every pixel live on a (partition, free) band of this tile.
  - ScalarE scales psum by 1/512 into a bf16 band tile; band DMAs back to
    DRAM (ACT HWDGE ring so input DMAs on the SP ring never stall).
  - Host extracts the 9 band diagonals (pure gather, no arithmetic
    beyond the device-computed means) into [9, 96, 96].
  - Inputs stream in 6 row-pair strips (x1 1.6MB + x2 ~1.6MB each) so PE
    chases the DMA; everything stays resident in SBUF (~150KB/partition).
"""

import os
import sys

for _p in ("/opt/trn_rl_repo",):
    if os.path.isdir(_p) and _p not in sys.path:
        sys.path.insert(0, _p)

from contextlib import ExitStack

import numpy as np
import ml_dtypes

import concourse.bass as bass
import concourse.mybir as mybir
import concourse.tile as tile
from concourse import bacc
from concourse.bass_utils import run_bass_kernel_spmd

B, C, H, W = 8, 512, 96, 96
NCORES = 8
NCT = C // 128           # 4 channel tiles
PH, PW = H + 2, W + 2    # 98x98 padded x2
_patch = os.environ.get("CORR_PATCH", "8x16")
PR, PC = (int(v) for v in _patch.split("x"))  # interior patch rows x cols
HR, HC = PR + 2, PC + 2  # halo 10x18 (N = 180)
NBI, NBJ = H // PR, W // PC   # 12 x 6 patch grid
NPAIR = NBI // 2         # 6 strip groups (2 bi-rows each)
PPG = 2 * NBJ            # patches per strip group = 12
MPIX = PR * PC           # 128
NHALO = HR * HC          # 180
F32 = mybir.dt.float32
BF16 = mybir.dt.bfloat16
BF16NP = ml_dtypes.bfloat16


def _corr_body(ctx: ExitStack, tc: "tile.TileContext", out_t, x1_t, x2_t):
    nc = tc.nc

    x1pool = ctx.enter_context(tc.tile_pool(name="x1", bufs=1))
    x2pool = ctx.enter_context(tc.tile_pool(name="x2", bufs=1))
    bdpool = ctx.enter_context(tc.tile_pool(name="bd", bufs=3))
    pspool = ctx.enter_context(
        tc.tile_pool(name="ps", bufs=8, space=bass.MemorySpace.PSUM)
    )

    SLABI = NCT * NBJ * MPIX  # x1 elems per bi-row strip = 3072
    x1t = x1pool.tile([128, NBI * SLABI], BF16)
    # h-major so each strip is one contiguous per-partition run (~6-12KB
    # descriptors on both DMA sides).
    x2t = x2pool.tile([128, PH, NCT, PW], BF16)

    x1f = x1_t.ap()  # [NBI, 128, NCT*NBJ*MPIX] bf16 DRAM (c-major per strip)
    x2f = x2_t.ap()  # [128, PH, NCT, PW] bf16 DRAM
    outf = out_t.ap()  # [NPAIR, MPIX, PPG, NHALO] bf16 DRAM

    # Input strips, interleaved x1/x2 per bi-row so PE chases the DMA with
    # idle gaps well under the ~3.4us HAM re-throttle window.
    # x2 strip bi covers padded rows row0[bi]..row0[bi+1] (10 rows, then 8).
    row0 = [0] + [PR * (bi + 1) + 2 for bi in range(NBI)]
    for bi in range(NBI):
        base = bi * SLABI
        nc.sync.dma_start(out=x1t[:, base : base + SLABI], in_=x1f[bi])
        r0, r1 = row0[bi], row0[bi + 1]
        nc.sync.dma_start(out=x2t[:, r0:r1, :, :], in_=x2f[:, r0:r1, :, :])

    inv = 1.0 / C
    for b in range(NPAIR):
        band = bdpool.tile([MPIX, PPG, NHALO], BF16, name="band")
        for q in range(PPG):
            halfbi, bj = divmod(q, NBJ)
            bi = 2 * b + halfbi
            ps = pspool.tile([MPIX, NHALO], F32, name="ps")
            for t in range(NCT):
                lbase = ((bi * NCT + t) * NBJ + bj) * MPIX
                nc.tensor.matmul(
                    ps[:, :],
                    x1t[:, lbase : lbase + MPIX],
                    x2t[:, PR * bi : PR * bi + HR, t, PC * bj : PC * bj + HC],
                    start=(t == 0),
                    stop=(t == NCT - 1),
                )
            nc.scalar.mul(band[:, q, :], ps[:, :], inv)
        # ACT HWDGE ring: naturally ordered after this band's ScalarE muls,
        # never stalls the SP input-DMA ring. Two halves so the first can
        # stream out while the second bi-row's patches are still copying.
        nc.scalar.dma_start(out=outf[b, :, : PPG // 2], in_=band[:, : PPG // 2, :])
        nc.scalar.dma_start(out=outf[b, :, PPG // 2 :], in_=band[:, PPG // 2 :, :])


_CACHE = {}


def _build(debug=False):
    key = ("nc", _patch)
    if key in _CACHE:
        return _CACHE[key]
    nc = bacc.Bacc("TRN2", target_bir_lowering=False, debug=debug)
    x1_t = nc.dram_tensor(
        "x1w", [NBI, 128, NCT * NBJ * MPIX], BF16, kind="ExternalInput"
    )
    x2_t = nc.dram_tensor("x2p", [128, PH, NCT, PW], BF16, kind="ExternalInput")
    out_t = nc.dram_tensor(
        "out", [NPAIR, MPIX, PPG, NHALO], BF16, kind="ExternalOutput"
    )
    with tile.TileContext(nc) as tc, ExitStack() as ctx:
        _corr_body(ctx, tc, out_t, x1_t, x2_t)
    nc.compile()
    _CACHE[key] = nc
    return nc


def prep_core_inputs(x1b: np.ndarray, x2b: np.ndarray) -> dict:
    """Pack one sample's fp32 [C,H,W] pair into the device layouts."""
    a = x1b.astype(BF16NP).reshape(NCT, 128, NBI, PR, NBJ, PC)
    # -> [bi, c, ct, bj, r, s]: per (bi, c) one contiguous slab
    a = np.ascontiguousarray(a.transpose(2, 1, 0, 4, 3, 5))
    x1w = a.reshape(NBI, 128, NCT * NBJ * MPIX)
    x2p = np.zeros((128, PH, NCT, PW), BF16NP)
    x2p[:, 1 : H + 1, :, 1 : W + 1] = (
        x2b.astype(BF16NP).reshape(NCT, 128, H, W).transpose(1, 2, 0, 3)
    )
    return {"x1w": x1w, "x2p": x2p}


_RR, _SS = np.mgrid[0:PR, 0:PC]


def extract_band(band: np.ndarray) -> np.ndarray:
    """[NPAIR, MPIX, PPG, NHALO] band (already scaled by 1/C) -> [9, H, W]."""
    v = np.asarray(band).astype(np.float32)
    v = v.reshape(NPAIR, PR, PC, 2, NBJ, NHALO)  # [pair, r, s, halfbi, bj, n]
    out = np.empty((9, H, W), np.float32)
    for dx in range(3):
        for dy in range(3):
            n = HC * (_RR + dy) + _SS + dx  # [PR, PC]
            g = v[:, _RR, _SS, :, :, n]  # -> [r, s, pair, halfbi, bj]
            out[3 * dx + dy] = (
                g.transpose(2, 3, 0, 4, 1).reshape(H, W)
            )
    return out


def kernel(x_1: np.ndarray, x_2: np.ndarray) -> np.ndarray:
    x_1 = np.ascontiguousarray(np.asarray(x_1), dtype=np.float32)
    x_2 = np.ascontiguousarray(np.asarray(x_2), dtype=np.float32)
    assert x_1.shape == (B, C, H, W) and x_2.shape == (B, C, H, W)
    nc = _build()
    in_maps = [prep_core_inputs(x_1[i], x_2[i]) for i in range(NCORES)]
    last_err = None
    for attempt in range(3):
        try:
            res = run_bass_kernel_spmd(nc, in_maps, list(range(NCORES)))
            out = np.stack(
                [extract_band(res.results[i]["out"]) for i in range(NCORES)],
                axis=0,
            )
            return out.astype(np.float32)
        except Exception as e:  # rare transient device faults - retry
            last_err = e
            import time as _time

            _time.sleep(5.0 * (attempt + 1))
    raise last_err


if __name__ == "__main__":
    rng = np.random.default_rng(0)
    a = rng.standard_normal((B, C, H, W), dtype=np.float32)
    b = rng.standard_normal((B, C, H, W), dtype=np.float32)
    o = kernel(a, b)
    print("out", o.shape, o.dtype, float(np.abs(o).max()))
